# revision 12
# baseline (speedup 1.0000x reference)
"""Trainium2 Bass kernel for causal+padded multi-head attention.

Problem: B=2, N=2048, D=1024, H=16 heads (DK=64), fp32 I/O.
  out = softmax(mask(x Wq^T (x Wk^T)^T) / sqrt(DK)) (x Wv^T) Wout^T + b_out

Sharding (8 cores): core c handles batch b=c//4 and heads [4*(c%4), 4*(c%4)+4).
Each core computes a partial output [N, D] (its 4 heads' contribution through
the output projection); the host sums the 4 partials per batch and adds b_out.

On-device layout (per core):
  xT   [1024, 2048]  (host-pretransposed x[b])
  QT/KT stored transposed [dk, n] as head-pair tiles [128, 2048]
  V    stored natural as [128(keys), 16 blocks, 4 heads, 65] with a ones
       column appended (col 64) so P@V' also yields the softmax denominator.
  S^T  computed per (head, q-tile 512, key-block 128) as [128, 512] in PSUM:
       matmul(lhsT=KT slice [64,128], rhs=QT slice [64,512]).
       Causal/pad masking = additive -30000 on PSUM (DVE), then one fused
       exp(0.125*s) on ScalarE straight into SBUF as float32r.
  ctx'^T [65, 512] accumulated in PSUM over key blocks:
       matmul(lhsT=V' [128,65], rhs=P^T [128,512]).
  Normalization: r = recip(rowsum) on the [1,512] denominator row,
       partition-broadcast to [64,512] (GpSimd), one DVE multiply.
  Out projection: matmul(lhsT=ctxT [128,128], rhs=WoutT [128,512]) acc over
       the two head-pair chunks.

All matmul operands are float32r (fp32 storage, ~11-bit-mantissa multiply,
full PE rate); accumulation is fp32 in PSUM.
"""

import math
import os

import numpy as np

B, N, D, H = 2, 2048, 1024, 16
DK = D // H  # 64
NCORES = 8
HEADS_PER_CORE = 4
QTILE = 512
KBLK = 128
NEG = -30000.0
NEGB = -3750.0  # pad bias applied after the 0.125 scale inside exp
SCALE = 1.0 / math.sqrt(float(DK))  # 0.125

# Set by run() when tracing is enabled (test.py reads this).
LAST_RESULTS = None


def _build_program(kb_max: int, jpad_min: int):
    import concourse.tile as tile
    from concourse import bacc, mybir

    F32 = mybir.dt.float32
    F32R = mybir.dt.float32r
    BF16 = mybir.dt.bfloat16
    EXP = mybir.ActivationFunctionType.Exp
    ADD = mybir.AluOpType.add

    nc = bacc.Bacc(None)

    xt_d = nc.dram_tensor("xt", [D, N], F32R, kind="ExternalInput")
    wq_d = nc.dram_tensor("wq", [D, 256], F32R, kind="ExternalInput")
    wk_d = nc.dram_tensor("wk", [D, 256], F32R, kind="ExternalInput")
    wv_d = nc.dram_tensor("wv", [D, 256], F32R, kind="ExternalInput")
    wout_d = nc.dram_tensor("wout", [256, D], F32R, kind="ExternalInput")
    padb_d = nc.dram_tensor("padbias", [128, 16], F32, kind="ExternalInput")
    trineg_d = nc.dram_tensor("trineg", [128, 896], F32, kind="ExternalInput")
    ones_d = nc.dram_tensor("ones65", [128, 64], BF16, kind="ExternalInput")
    out_d = nc.dram_tensor("out", [N, D], F32, kind="ExternalOutput")

    NB = N // KBLK  # 16 key/row blocks
    NQT = N // QTILE  # 4 q tiles

    with tile.TileContext(nc) as tc:
        with (
            tc.tile_pool(name="xt", bufs=8) as xt_pool,
            tc.tile_pool(name="w", bufs=1) as w_pool,
            tc.tile_pool(name="big", bufs=1) as big_pool,
            tc.tile_pool(name="work", bufs=2) as work_pool,
            tc.tile_pool(name="pt", bufs=3) as pt_pool,
            tc.tile_pool(name="ps_proj", bufs=2, space="PSUM") as ps_proj,
            tc.tile_pool(name="ps_st", bufs=2, space="PSUM") as ps_st,
            tc.tile_pool(name="ps_ctx", bufs=2, space="PSUM") as ps_ctx,
        ):
            # ---- load inputs ----
            xt = []
            for e in range(8):
                t = xt_pool.tile([128, N], F32R, tag="xt")
                nc.sync.dma_start(t[:], xt_d[e * 128:(e + 1) * 128, :])
                xt.append(t)
            wq_t = w_pool.tile([128, 8, 256], F32R, tag="wq")
            wk_t = w_pool.tile([128, 8, 256], F32R, tag="wk")
            wv_t = w_pool.tile([128, 8, 256], F32R, tag="wv")
            wo_t = w_pool.tile([128, 2, D], F32R, tag="wo")
            nc.sync.dma_start(wq_t[:], wq_d[:].rearrange("(e p) m -> p e m", p=128))
            nc.sync.dma_start(wk_t[:], wk_d[:].rearrange("(e p) m -> p e m", p=128))
            nc.sync.dma_start(wv_t[:], wv_d[:].rearrange("(e p) m -> p e m", p=128))
            nc.sync.dma_start(wo_t[:], wout_d[:].rearrange("(c p) m -> p c m", p=128))
            padb_t = w_pool.tile([128, 16], F32, tag="padb")
            trineg_t = w_pool.tile([128, 896], F32, tag="trineg")
            nc.sync.dma_start(padb_t[:], padb_d[:])
            nc.sync.dma_start(trineg_t[:], trineg_d[:])

            # V' tile: [keys 128, key-block 16, head 4, 65]; col 64 <- ones
            v4 = big_pool.tile([128, NB, 4, 65], BF16, tag="v4")
            nc.sync.dma_start(
                v4[:, :, :, 64:65],
                ones_d[:].rearrange("p (b h o) -> p b h o", h=4, o=1),
            )

            heat_w = w_pool.tile([1, 64], BF16, tag="heatw")
            nc.vector.memset(heat_w[:], 0.0)

            def heater():
                # fp32-HIGH matmuls do not register as PE activity for the
                # HAM clock gate; a tiny bf16 matmul does, keeping the PE
                # at 2.4 GHz through the f32r-only stretches.
                ht = ps_ctx.tile([1, 64], F32, tag="ctx", name="heat")
                nc.tensor.matmul(
                    ht[:], heat_w[:, 0:1], heat_w[:], start=True, stop=True
                )

            qt_pair = [big_pool.tile([128, N], F32R, tag=f"qt{p}", name=f"qt{p}") for p in range(2)]
            kt_pair = [big_pool.tile([128, N], F32R, tag=f"kt{p}", name=f"kt{p}") for p in range(2)]
            ctx_pair = [big_pool.tile([128, N], F32R, tag=f"ctx{p}", name=f"ctx{p}") for p in range(2)]

            # ---- phase B: projections ----
            # QT/KT: [dk(128 = 2 heads), n] = (W.T chunk)^T @ xT
            for name, w_t, dst in (("q", wq_t, qt_pair), ("k", wk_t, kt_pair)):
                for pair in range(2):
                    for nq in range(NQT):
                        ps = ps_proj.tile([128, 512], F32, tag="proj")
                        for e in range(8):
                            nc.tensor.matmul(
                                ps[:],
                                wq_t[:, e, pair * 128:(pair + 1) * 128]
                                if name == "q"
                                else wk_t[:, e, pair * 128:(pair + 1) * 128],
                                xt[e][:, nq * 512:(nq + 1) * 512],
                                start=(e == 0),
                                stop=(e == 7),
                            )
                        nc.vector.tensor_copy(
                            dst[pair][:, nq * 512:(nq + 1) * 512], ps[:]
                        )
                        heater()
            # V natural: [n-block, 4*64] = xT-chunk^T @ WvT-chunk
            for nb in range(NB):
                ps = ps_proj.tile([128, 256], F32, tag="proj")
                for e in range(8):
                    nc.tensor.matmul(
                        ps[:],
                        xt[e][:, nb * 128:(nb + 1) * 128],
                        wv_t[:, e, :],
                        start=(e == 0),
                        stop=(e == 7),
                    )
                nc.vector.tensor_copy(
                    v4[:, nb, :, 0:64],
                    ps[:].rearrange("p (h d) -> p h d", h=4),
                )
                heater()

            # ---- phase C: attention per head ----
            # Normalization is software-pipelined one (head, q-tile) unit
            # behind the chunk loop so the DVE reciprocal (the slowest DVE
            # op) never sits between the mask-adds the PE is waiting on.
            def emit_normalize(h, qt, ctx_ps):
                pair, hh = divmod(h, 2)
                hp = slice(64 * hh, 64 * hh + 64)
                craw = work_pool.tile([65, 512], F32, tag="craw", name="craw")
                nc.scalar.copy(craw[:], ctx_ps[:])
                rrec = work_pool.tile([1, 512], F32, tag="rrec", name="rrec")
                nc.vector.reciprocal(rrec[:], craw[64:65, :])
                rbr = work_pool.tile([64, 512], F32, tag="rbr", name="rbr")
                nc.gpsimd.partition_broadcast(rbr[:], rrec[:])
                nc.vector.tensor_mul(
                    ctx_pair[pair][hp, qt * 512:(qt + 1) * 512],
                    craw[0:64, :],
                    rbr[:],
                )

            pending = None
            for h in range(HEADS_PER_CORE):
                pair, hh = divmod(h, 2)
                hp = slice(64 * hh, 64 * hh + 64)
                for qt in range(NQT):
                    nchunks = min(4 * qt + 4, kb_max)
                    ctx_ps = ps_ctx.tile([65, 512], F32, tag="ctx", name="ctx")
                    npairs = (nchunks + 1) // 2
                    for pr in range(npairs):
                        sub = min(2, nchunks - 2 * pr)
                        st_ps = ps_st.tile([128, 2, 512], F32, tag="st")
                        offs = []
                        for s in range(sub):
                            j = 2 * pr + s
                            d = j - 4 * qt
                            # exact-causal column trim (keep matmul N >= 256)
                            off = min(128 * d, 256) if d >= 1 else 0
                            offs.append(off)
                            nc.tensor.matmul(
                                st_ps[:, s, off:],
                                kt_pair[pair][hp, j * 128:(j + 1) * 128],
                                qt_pair[pair][hp, qt * 512 + off:(qt + 1) * 512],
                                start=True,
                                stop=True,
                            )
                            if d >= 0:  # diagonal block: causal add -30000
                                u0 = 384 - 128 * d + off
                                nc.vector.tensor_tensor(
                                    st_ps[:, s, off:],
                                    st_ps[:, s, off:],
                                    trineg_t[:, u0:u0 + 512 - off],
                                    ADD,
                                )
                        pt_t = pt_pool.tile([128, 2, 512], BF16, tag="pt")
                        j0 = 2 * pr
                        if sub == 2 and offs[0] == offs[1] and j0 + 1 < jpad_min:
                            nc.scalar.activation(
                                pt_t[:, :, offs[0]:],
                                st_ps[:, :, offs[0]:],
                                EXP,
                                scale=SCALE,
                            )
                        else:
                            for s in range(sub):
                                j = j0 + s
                                kw = {}
                                if j >= jpad_min:
                                    kw["bias"] = padb_t[:, j:j + 1]
                                nc.scalar.activation(
                                    pt_t[:, s, offs[s]:],
                                    st_ps[:, s, offs[s]:],
                                    EXP,
                                    scale=SCALE,
                                    **kw,
                                )
                        for s in range(sub):
                            j = j0 + s
                            nc.tensor.matmul(
                                ctx_ps[:, offs[s]:],
                                v4[:, j, h, :],
                                pt_t[:, s, offs[s]:],
                                start=(j == 0),
                                stop=(j == nchunks - 1),
                                skip_group_check=True,
                            )
                    if pending is not None:
                        emit_normalize(*pending)
                    pending = (h, qt, ctx_ps)
            emit_normalize(*pending)

            # ---- phase D: output projection ----
            for nb in range(NB):
                osb = work_pool.tile([128, D], F32, tag="osb")
                for fc in range(2):
                    ps = ps_proj.tile([128, 512], F32, tag="proj")
                    for pair in range(2):
                        nc.tensor.matmul(
                            ps[:],
                            ctx_pair[pair][:, nb * 128:(nb + 1) * 128],
                            wo_t[:, pair, fc * 512:(fc + 1) * 512],
                            start=(pair == 0),
                            stop=(pair == 1),
                        )
                    nc.vector.tensor_copy(osb[:, fc * 512:(fc + 1) * 512], ps[:])
                heater()
                nc.sync.dma_start(out_d[nb * 128:(nb + 1) * 128, :], osb[:])

    nc.compile()
    return nc


_PROGRAM_CACHE = {}


def kernel(x, attention_mask, W_Q, W_K, W_V, W_out, b_out):
    global LAST_RESULTS
    from concourse.bass_utils import run_bass_kernel_spmd

    x = np.ascontiguousarray(x, dtype=np.float32)
    attention_mask = np.asarray(attention_mask)
    lengths = attention_mask.astype(np.int64).sum(axis=1)
    kb_max = int(math.ceil(lengths.max() / KBLK))
    jpad_min = int(lengths.min() // KBLK)

    key = (kb_max, jpad_min)
    if key not in _PROGRAM_CACHE:
        _PROGRAM_CACHE[key] = _build_program(kb_max, jpad_min)
    nc = _PROGRAM_CACHE[key]

    # host-side input prep
    xT = [np.ascontiguousarray(x[b].T) for b in range(B)]
    wqT = np.ascontiguousarray(np.asarray(W_Q, dtype=np.float32).T)
    wkT = np.ascontiguousarray(np.asarray(W_K, dtype=np.float32).T)
    wvT = np.ascontiguousarray(np.asarray(W_V, dtype=np.float32).T)
    woT = np.ascontiguousarray(np.asarray(W_out, dtype=np.float32).T)
    # padbias[p, j] = 0 if key j*128+p is real else -30000
    padb = [
        np.ascontiguousarray(
            np.where(attention_mask[b].reshape(16, 128).T != 0, 0.0, NEGB)
        ).astype(np.float32)
        for b in range(B)
    ]
    # trineg[p, u] = NEG if u < p + 384 else 0; slice [384-128d : 896-128d]
    # gives the causal additive mask for a diagonal block with offset 128d.
    pp = np.arange(128)[:, None]
    uu = np.arange(896)[None, :]
    trineg = np.where(uu < pp + 384, NEG, 0.0).astype(np.float32)
    import ml_dtypes
    ones65 = np.ones((128, 64), dtype=ml_dtypes.bfloat16)

    in_maps = []
    for c in range(NCORES):
        b, g = divmod(c, 4)
        sl = slice(g * 256, (g + 1) * 256)
        in_maps.append(
            {
                "xt": xT[b],
                "wq": np.ascontiguousarray(wqT[:, sl]),
                "wk": np.ascontiguousarray(wkT[:, sl]),
                "wv": np.ascontiguousarray(wvT[:, sl]),
                "wout": np.ascontiguousarray(woT[sl, :]),
                "padbias": padb[b],
                "trineg": trineg,
                "ones65": ones65,
            }
        )

    trace = bool(int(os.environ.get("KERNEL_TRACE", "0")))
    res = run_bass_kernel_spmd(
        nc,
        in_maps,
        core_ids=list(range(NCORES)),
        trace=trace,
        trace_cores=list(range(NCORES)) if trace else None,
    )
    LAST_RESULTS = res

    out = np.zeros((B, N, D), dtype=np.float32)
    for c in range(NCORES):
        out[c // 4] += res.results[c]["out"]
    out += np.asarray(b_out, dtype=np.float32)[None, None, :]
    return out


# revision 15
# speedup vs baseline: 1.0147x; 1.0147x over previous
"""Trainium2 Bass kernel for causal+padded multi-head attention.

Problem: B=2, N=2048, D=1024, H=16 heads (DK=64), fp32 I/O.
  out = softmax(mask(x Wq^T (x Wk^T)^T) / sqrt(DK)) (x Wv^T) Wout^T + b_out

Sharding (8 cores): core c handles batch b=c//4 and heads [4*(c%4), 4*(c%4)+4).
Each core computes a partial output [N, D] (its 4 heads' contribution through
the output projection); the host sums the 4 partials per batch and adds b_out.

On-device layout (per core):
  xT   [1024, 2048]  (host-pretransposed x[b])
  QT/KT stored transposed [dk, n] as head-pair tiles [128, 2048]
  V    stored natural as [128(keys), 16 blocks, 4 heads, 65] with a ones
       column appended (col 64) so P@V' also yields the softmax denominator.
  S^T  computed per (head, q-tile 512, key-block 128) as [128, 512] in PSUM:
       matmul(lhsT=KT slice [64,128], rhs=QT slice [64,512]).
       Causal/pad masking = additive -30000 on PSUM (DVE), then one fused
       exp(0.125*s) on ScalarE straight into SBUF as float32r.
  ctx'^T [65, 512] accumulated in PSUM over key blocks:
       matmul(lhsT=V' [128,65], rhs=P^T [128,512]).
  Normalization: r = recip(rowsum) on the [1,512] denominator row,
       partition-broadcast to [64,512] (GpSimd), one DVE multiply.
  Out projection: matmul(lhsT=ctxT [128,128], rhs=WoutT [128,512]) acc over
       the two head-pair chunks.

All matmul operands are float32r (fp32 storage, ~11-bit-mantissa multiply,
full PE rate); accumulation is fp32 in PSUM.
"""

import math
import os

import numpy as np

B, N, D, H = 2, 2048, 1024, 16
DK = D // H  # 64
NCORES = 8
HEADS_PER_CORE = 4
QTILE = 512
KBLK = 128
NEG = -30000.0
NEGB = -3750.0  # pad bias applied after the 0.125 scale inside exp
SCALE = 1.0 / math.sqrt(float(DK))  # 0.125

# Set by run() when tracing is enabled (test.py reads this).
LAST_RESULTS = None


def _build_program(kb_max: int, jpad_min: int):
    import concourse.tile as tile
    from concourse import bacc, mybir

    F32 = mybir.dt.float32
    F32R = mybir.dt.float32r
    BF16 = mybir.dt.bfloat16
    EXP = mybir.ActivationFunctionType.Exp
    ADD = mybir.AluOpType.add

    nc = bacc.Bacc(None)

    xt_d = nc.dram_tensor("xt", [D, N], F32R, kind="ExternalInput")
    wq_d = nc.dram_tensor("wq", [D, 256], F32R, kind="ExternalInput")
    wk_d = nc.dram_tensor("wk", [D, 256], F32R, kind="ExternalInput")
    wv_d = nc.dram_tensor("wv", [D, 256], F32R, kind="ExternalInput")
    wout_d = nc.dram_tensor("wout", [256, D], F32R, kind="ExternalInput")
    padb_d = nc.dram_tensor("padbias", [128, 16], F32, kind="ExternalInput")
    trineg_d = nc.dram_tensor("trineg", [128, 896], F32, kind="ExternalInput")
    ones_d = nc.dram_tensor("ones65", [128, 64], BF16, kind="ExternalInput")
    out_d = nc.dram_tensor("out", [N, D], F32, kind="ExternalOutput")

    NB = N // KBLK  # 16 key/row blocks
    NQT = N // QTILE  # 4 q tiles

    with tile.TileContext(nc) as tc:
        with (
            tc.tile_pool(name="w", bufs=1) as w_pool,
            tc.tile_pool(name="big", bufs=1) as big_pool,
            tc.tile_pool(name="work", bufs=2) as work_pool,
            tc.tile_pool(name="ps_proj", bufs=2, space="PSUM") as ps_proj,
            tc.tile_pool(name="ps_st", bufs=2, space="PSUM") as ps_st,
            tc.tile_pool(name="ps_ctx", bufs=2, space="PSUM") as ps_ctx,
        ):
            # ---- load inputs ----
            xt_cm = tc.tile_pool(name="xt", bufs=8)
            xt_pool = xt_cm.__enter__()
            xt = []
            for e in range(8):
                t = xt_pool.tile([128, N], F32R, tag="xt")
                nc.sync.dma_start(t[:], xt_d[e * 128:(e + 1) * 128, :])
                xt.append(t)
            wq_t = w_pool.tile([128, 8, 256], F32R, tag="wq")
            wk_t = w_pool.tile([128, 8, 256], F32R, tag="wk")
            wv_t = w_pool.tile([128, 8, 256], F32R, tag="wv")
            wo_t = w_pool.tile([128, 2, D], F32R, tag="wo")
            nc.sync.dma_start(wq_t[:], wq_d[:].rearrange("(e p) m -> p e m", p=128))
            nc.sync.dma_start(wk_t[:], wk_d[:].rearrange("(e p) m -> p e m", p=128))
            nc.sync.dma_start(wv_t[:], wv_d[:].rearrange("(e p) m -> p e m", p=128))
            nc.sync.dma_start(wo_t[:], wout_d[:].rearrange("(c p) m -> p c m", p=128))
            padb_t = w_pool.tile([128, 16], F32, tag="padb")
            trineg_t = w_pool.tile([128, 896], F32, tag="trineg")
            nc.sync.dma_start(padb_t[:], padb_d[:])
            nc.sync.dma_start(trineg_t[:], trineg_d[:])

            # V' tile: [keys 128, key-block 16, head 4, 65]; col 64 <- ones
            v4 = big_pool.tile([128, NB, 4, 65], BF16, tag="v4")
            nc.sync.dma_start(
                v4[:, :, :, 64:65],
                ones_d[:].rearrange("p (b h o) -> p b h o", h=4, o=1),
            )

            heat_w = w_pool.tile([1, 64], BF16, tag="heatw")
            nc.vector.memset(heat_w[:], 0.0)

            def heater():
                # fp32-HIGH matmuls do not register as PE activity for the
                # HAM clock gate; a tiny bf16 matmul does, keeping the PE
                # at 2.4 GHz through the f32r-only stretches.
                ht = ps_ctx.tile([1, 64], F32, tag="ctx", name="heat")
                nc.tensor.matmul(
                    ht[:], heat_w[:, 0:1], heat_w[:], start=True, stop=True
                )

            qt_pair = [big_pool.tile([128, N], F32R, tag=f"qt{p}", name=f"qt{p}") for p in range(2)]
            kt_pair = [big_pool.tile([128, N], F32R, tag=f"kt{p}", name=f"kt{p}") for p in range(2)]
            ctx_pair = [big_pool.tile([128, N], F32R, tag=f"ctx{p}", name=f"ctx{p}") for p in range(2)]

            # ---- phase B: projections ----
            # QT/KT: [dk(128 = 2 heads), n] = (W.T chunk)^T @ xT
            for name, w_t, dst in (("q", wq_t, qt_pair), ("k", wk_t, kt_pair)):
                for pair in range(2):
                    for nq in range(NQT):
                        ps = ps_proj.tile([128, 512], F32, tag="proj")
                        for e in range(8):
                            nc.tensor.matmul(
                                ps[:],
                                wq_t[:, e, pair * 128:(pair + 1) * 128]
                                if name == "q"
                                else wk_t[:, e, pair * 128:(pair + 1) * 128],
                                xt[e][:, nq * 512:(nq + 1) * 512],
                                start=(e == 0),
                                stop=(e == 7),
                            )
                        nc.vector.tensor_copy(
                            dst[pair][:, nq * 512:(nq + 1) * 512], ps[:]
                        )
                        heater()
            # V natural: [n-block, 4*64] = xT-chunk^T @ WvT-chunk
            for nb in range(NB):
                ps = ps_proj.tile([128, 256], F32, tag="proj")
                for e in range(8):
                    nc.tensor.matmul(
                        ps[:],
                        xt[e][:, nb * 128:(nb + 1) * 128],
                        wv_t[:, e, :],
                        start=(e == 0),
                        stop=(e == 7),
                    )
                nc.vector.tensor_copy(
                    v4[:, nb, :, 0:64],
                    ps[:].rearrange("p (h d) -> p h d", h=4),
                )
                heater()

            xt_cm.__exit__(None, None, None)
            pt_cm = tc.tile_pool(name="pt", bufs=14)
            pt_pool = pt_cm.__enter__()

            # ---- phase C: attention per head ----
            # Normalization is software-pipelined one (head, q-tile) unit
            # behind the chunk loop so the DVE reciprocal (the slowest DVE
            # op) never sits between the mask-adds the PE is waiting on.
            def emit_normalize(h, qt, ctx_ps):
                pair, hh = divmod(h, 2)
                hp = slice(64 * hh, 64 * hh + 64)
                craw = work_pool.tile([65, 512], F32, tag="craw", name="craw")
                nc.scalar.copy(craw[:], ctx_ps[:])
                rrec = work_pool.tile([1, 512], F32, tag="rrec", name="rrec")
                nc.vector.reciprocal(rrec[:], craw[64:65, :])
                rbr = work_pool.tile([64, 512], F32, tag="rbr", name="rbr")
                nc.gpsimd.partition_broadcast(rbr[:], rrec[:])
                nc.vector.tensor_mul(
                    ctx_pair[pair][hp, qt * 512:(qt + 1) * 512],
                    craw[0:64, :],
                    rbr[:],
                )

            # The PV matmuls for a unit are emitted while the NEXT unit's
            # S^T matmuls run, so by the time the PE (in-order) reaches a
            # PV, its exp finished long ago — the PE never drains waiting
            # on ScalarE, which would re-throttle the HAM clock gate.
            def emit_st_exp(h, qt, nchunks):
                """S^T + mask + exp for all chunks; returns PV descriptors."""
                pair, hh = divmod(h, 2)
                hp = slice(64 * hh, 64 * hh + 64)
                pv = []
                for pr in range((nchunks + 1) // 2):
                    sub = min(2, nchunks - 2 * pr)
                    st_ps = ps_st.tile([128, 2, 512], F32, tag="st")
                    offs = []
                    for s in range(sub):
                        j = 2 * pr + s
                        d = j - 4 * qt
                        # exact-causal column trim (keep matmul N >= 256)
                        off = min(128 * d, 256) if d >= 1 else 0
                        offs.append(off)
                        nc.tensor.matmul(
                            st_ps[:, s, off:],
                            kt_pair[pair][hp, j * 128:(j + 1) * 128],
                            qt_pair[pair][hp, qt * 512 + off:(qt + 1) * 512],
                            start=True,
                            stop=True,
                        )
                        if d >= 0:  # diagonal block: causal add -30000
                            u0 = 384 - 128 * d + off
                            nc.vector.tensor_tensor(
                                st_ps[:, s, off:],
                                st_ps[:, s, off:],
                                trineg_t[:, u0:u0 + 512 - off],
                                ADD,
                            )
                    pt_t = pt_pool.tile([128, 2, 512], BF16, tag="pt")
                    j0 = 2 * pr
                    if sub == 2 and offs[0] == offs[1] and j0 + 1 < jpad_min:
                        nc.scalar.activation(
                            pt_t[:, :, offs[0]:],
                            st_ps[:, :, offs[0]:],
                            EXP,
                            scale=SCALE,
                        )
                    else:
                        for s in range(sub):
                            j = j0 + s
                            kw = {}
                            if j >= jpad_min:
                                kw["bias"] = padb_t[:, j:j + 1]
                            nc.scalar.activation(
                                pt_t[:, s, offs[s]:],
                                st_ps[:, s, offs[s]:],
                                EXP,
                                scale=SCALE,
                                **kw,
                            )
                    for s in range(sub):
                        pv.append((j0 + s, pt_t, s, offs[s]))
                return pv

            def emit_pv(h, qt, nchunks, pv, ctx_ps):
                for j, pt_t, s, off in pv:
                    nc.tensor.matmul(
                        ctx_ps[:, off:],
                        v4[:, j, h, :],
                        pt_t[:, s, off:],
                        start=(j == 0),
                        stop=(j == nchunks - 1),
                        skip_group_check=True,
                    )

            units = [
                (h, qt, min(4 * qt + 4, kb_max))
                for h in range(HEADS_PER_CORE)
                for qt in range(NQT)
            ]
            prev_pv = None  # (h, qt, nchunks, pv_descs, ctx_ps)
            norm_q = []  # normalize two units behind
            for h, qt, nchunks in units:
                pv = emit_st_exp(h, qt, nchunks)
                if prev_pv is not None:
                    ph, pqt, pn, ppv, pctx = prev_pv
                    emit_pv(ph, pqt, pn, ppv, pctx)
                    norm_q.append((ph, pqt, pctx))
                if len(norm_q) > 1:
                    emit_normalize(*norm_q.pop(0))
                ctx_ps = ps_ctx.tile([65, 512], F32, tag="ctx", name="ctx")
                prev_pv = (h, qt, nchunks, pv, ctx_ps)
            ph, pqt, pn, ppv, pctx = prev_pv
            emit_pv(ph, pqt, pn, ppv, pctx)
            norm_q.append((ph, pqt, pctx))
            for u in norm_q:
                emit_normalize(*u)

            pt_cm.__exit__(None, None, None)

            # ---- phase D: output projection ----
            for nb in range(NB):
                osb = work_pool.tile([128, D], F32, tag="osb")
                for fc in range(2):
                    ps = ps_proj.tile([128, 512], F32, tag="proj")
                    for pair in range(2):
                        nc.tensor.matmul(
                            ps[:],
                            ctx_pair[pair][:, nb * 128:(nb + 1) * 128],
                            wo_t[:, pair, fc * 512:(fc + 1) * 512],
                            start=(pair == 0),
                            stop=(pair == 1),
                        )
                    nc.vector.tensor_copy(osb[:, fc * 512:(fc + 1) * 512], ps[:])
                heater()
                nc.sync.dma_start(out_d[nb * 128:(nb + 1) * 128, :], osb[:])

    nc.compile()
    return nc


_PROGRAM_CACHE = {}


def kernel(x, attention_mask, W_Q, W_K, W_V, W_out, b_out):
    global LAST_RESULTS
    from concourse.bass_utils import run_bass_kernel_spmd

    x = np.ascontiguousarray(x, dtype=np.float32)
    attention_mask = np.asarray(attention_mask)
    lengths = attention_mask.astype(np.int64).sum(axis=1)
    kb_max = int(math.ceil(lengths.max() / KBLK))
    jpad_min = int(lengths.min() // KBLK)

    key = (kb_max, jpad_min)
    if key not in _PROGRAM_CACHE:
        _PROGRAM_CACHE[key] = _build_program(kb_max, jpad_min)
    nc = _PROGRAM_CACHE[key]

    # host-side input prep
    xT = [np.ascontiguousarray(x[b].T) for b in range(B)]
    wqT = np.ascontiguousarray(np.asarray(W_Q, dtype=np.float32).T)
    wkT = np.ascontiguousarray(np.asarray(W_K, dtype=np.float32).T)
    wvT = np.ascontiguousarray(np.asarray(W_V, dtype=np.float32).T)
    woT = np.ascontiguousarray(np.asarray(W_out, dtype=np.float32).T)
    # padbias[p, j] = 0 if key j*128+p is real else -30000
    padb = [
        np.ascontiguousarray(
            np.where(attention_mask[b].reshape(16, 128).T != 0, 0.0, NEGB)
        ).astype(np.float32)
        for b in range(B)
    ]
    # trineg[p, u] = NEG if u < p + 384 else 0; slice [384-128d : 896-128d]
    # gives the causal additive mask for a diagonal block with offset 128d.
    pp = np.arange(128)[:, None]
    uu = np.arange(896)[None, :]
    trineg = np.where(uu < pp + 384, NEG, 0.0).astype(np.float32)
    import ml_dtypes
    ones65 = np.ones((128, 64), dtype=ml_dtypes.bfloat16)

    in_maps = []
    for c in range(NCORES):
        b, g = divmod(c, 4)
        sl = slice(g * 256, (g + 1) * 256)
        in_maps.append(
            {
                "xt": xT[b],
                "wq": np.ascontiguousarray(wqT[:, sl]),
                "wk": np.ascontiguousarray(wkT[:, sl]),
                "wv": np.ascontiguousarray(wvT[:, sl]),
                "wout": np.ascontiguousarray(woT[sl, :]),
                "padbias": padb[b],
                "trineg": trineg,
                "ones65": ones65,
            }
        )

    trace = bool(int(os.environ.get("KERNEL_TRACE", "0")))
    res = run_bass_kernel_spmd(
        nc,
        in_maps,
        core_ids=list(range(NCORES)),
        trace=trace,
        trace_cores=list(range(NCORES)) if trace else None,
    )
    LAST_RESULTS = res

    out = np.zeros((B, N, D), dtype=np.float32)
    for c in range(NCORES):
        out[c // 4] += res.results[c]["out"]
    out += np.asarray(b_out, dtype=np.float32)[None, None, :]
    return out


# revision 16
# speedup vs baseline: 1.1276x; 1.1113x over previous
"""Trainium2 Bass kernel for causal+padded multi-head attention.

Problem: B=2, N=2048, D=1024, H=16 heads (DK=64), fp32 I/O.
  out = softmax(mask(x Wq^T (x Wk^T)^T) / sqrt(DK)) (x Wv^T) Wout^T + b_out

Sharding (8 cores): core c handles batch b=c//4 and heads [4*(c%4), 4*(c%4)+4).
Each core computes a partial output [N, D] (its 4 heads' contribution through
the output projection); the host sums the 4 partials per batch and adds b_out.

On-device layout (per core):
  xT   [1024, 2048]  (host-pretransposed x[b])
  QT/KT stored transposed [dk, n] as head-pair tiles [128, 2048]
  V    stored natural as [128(keys), 16 blocks, 4 heads, 65] with a ones
       column appended (col 64) so P@V' also yields the softmax denominator.
  S^T  computed per (head, q-tile 512, key-block 128) as [128, 512] in PSUM:
       matmul(lhsT=KT slice [64,128], rhs=QT slice [64,512]).
       Causal/pad masking = additive -30000 on PSUM (DVE), then one fused
       exp(0.125*s) on ScalarE straight into SBUF as float32r.
  ctx'^T [65, 512] accumulated in PSUM over key blocks:
       matmul(lhsT=V' [128,65], rhs=P^T [128,512]).
  Normalization: r = recip(rowsum) on the [1,512] denominator row,
       partition-broadcast to [64,512] (GpSimd), one DVE multiply.
  Out projection: matmul(lhsT=ctxT [128,128], rhs=WoutT [128,512]) acc over
       the two head-pair chunks.

All matmul operands are float32r (fp32 storage, ~11-bit-mantissa multiply,
full PE rate); accumulation is fp32 in PSUM.
"""

import math
import os

import numpy as np

B, N, D, H = 2, 2048, 1024, 16
DK = D // H  # 64
NCORES = 8
HEADS_PER_CORE = 4
QTILE = 512
KBLK = 128
NEG = -30000.0
NEGB = -3750.0  # pad bias applied after the 0.125 scale inside exp
SCALE = 1.0 / math.sqrt(float(DK))  # 0.125

# Set by run() when tracing is enabled (test.py reads this).
LAST_RESULTS = None


def _build_program(kb_max: int, jpad_min: int):
    import concourse.tile as tile
    from concourse import bacc, mybir

    F32 = mybir.dt.float32
    F32R = mybir.dt.float32r
    BF16 = mybir.dt.bfloat16
    EXP = mybir.ActivationFunctionType.Exp
    ADD = mybir.AluOpType.add

    nc = bacc.Bacc(None)

    xt_d = nc.dram_tensor("xt", [D, N], BF16, kind="ExternalInput")
    wq_d = nc.dram_tensor("wq", [D, 256], BF16, kind="ExternalInput")
    wk_d = nc.dram_tensor("wk", [D, 256], BF16, kind="ExternalInput")
    wv_d = nc.dram_tensor("wv", [D, 256], BF16, kind="ExternalInput")
    wout_d = nc.dram_tensor("wout", [256, D], BF16, kind="ExternalInput")
    padb_d = nc.dram_tensor("padbias", [128, 16], F32, kind="ExternalInput")
    trineg_d = nc.dram_tensor("trineg", [128, 896], F32, kind="ExternalInput")
    ones_d = nc.dram_tensor("ones65", [128, 64], BF16, kind="ExternalInput")
    out_d = nc.dram_tensor("out", [N, D], F32, kind="ExternalOutput")

    NB = N // KBLK  # 16 key/row blocks
    NQT = N // QTILE  # 4 q tiles

    with tile.TileContext(nc) as tc:
        with (
            tc.tile_pool(name="w", bufs=1) as w_pool,
            tc.tile_pool(name="big", bufs=1) as big_pool,
            tc.tile_pool(name="work", bufs=2) as work_pool,
            tc.tile_pool(name="ps_proj", bufs=2, space="PSUM") as ps_proj,
            tc.tile_pool(name="ps_st", bufs=2, space="PSUM") as ps_st,
            tc.tile_pool(name="ps_ctx", bufs=2, space="PSUM") as ps_ctx,
        ):
            # ---- load inputs ----
            xt_cm = tc.tile_pool(name="xt", bufs=8)
            xt_pool = xt_cm.__enter__()
            xt = []
            for e in range(8):
                t = xt_pool.tile([128, N], BF16, tag="xt")
                nc.sync.dma_start(t[:], xt_d[e * 128:(e + 1) * 128, :])
                xt.append(t)
            wq_t = w_pool.tile([128, 8, 256], BF16, tag="wq")
            wk_t = w_pool.tile([128, 8, 256], BF16, tag="wk")
            wv_t = w_pool.tile([128, 8, 256], BF16, tag="wv")
            wo_t = w_pool.tile([128, 2, D], BF16, tag="wo")
            nc.sync.dma_start(wq_t[:], wq_d[:].rearrange("(e p) m -> p e m", p=128))
            nc.sync.dma_start(wk_t[:], wk_d[:].rearrange("(e p) m -> p e m", p=128))
            nc.sync.dma_start(wv_t[:], wv_d[:].rearrange("(e p) m -> p e m", p=128))
            nc.sync.dma_start(wo_t[:], wout_d[:].rearrange("(c p) m -> p c m", p=128))
            padb_t = w_pool.tile([128, 16], F32, tag="padb")
            trineg_t = w_pool.tile([128, 896], F32, tag="trineg")
            nc.sync.dma_start(padb_t[:], padb_d[:])
            nc.sync.dma_start(trineg_t[:], trineg_d[:])

            # V' tile: [keys 128, key-block 16, head 4, 65]; col 64 <- ones
            v4 = big_pool.tile([128, NB, 4, 65], BF16, tag="v4")
            nc.sync.dma_start(
                v4[:, :, :, 64:65],
                ones_d[:].rearrange("p (b h o) -> p b h o", h=4, o=1),
            )

            qt_pair = [big_pool.tile([128, N], BF16, tag=f"qt{p}", name=f"qt{p}") for p in range(2)]
            kt_pair = [big_pool.tile([128, N], BF16, tag=f"kt{p}", name=f"kt{p}") for p in range(2)]
            ctx_pair = [big_pool.tile([128, N], BF16, tag=f"ctx{p}", name=f"ctx{p}") for p in range(2)]

            # ---- phase B: projections ----
            # QT/KT: [dk(128 = 2 heads), n] = (W.T chunk)^T @ xT
            for name, w_t, dst in (("q", wq_t, qt_pair), ("k", wk_t, kt_pair)):
                for pair in range(2):
                    for nq in range(NQT):
                        ps = ps_proj.tile([128, 512], F32, tag="proj")
                        for e in range(8):
                            nc.tensor.matmul(
                                ps[:],
                                wq_t[:, e, pair * 128:(pair + 1) * 128]
                                if name == "q"
                                else wk_t[:, e, pair * 128:(pair + 1) * 128],
                                xt[e][:, nq * 512:(nq + 1) * 512],
                                start=(e == 0),
                                stop=(e == 7),
                            )
                        nc.vector.tensor_copy(
                            dst[pair][:, nq * 512:(nq + 1) * 512], ps[:]
                        )
            # V natural: [n-block, 4*64] = xT-chunk^T @ WvT-chunk
            for nb in range(NB):
                ps = ps_proj.tile([128, 256], F32, tag="proj")
                for e in range(8):
                    nc.tensor.matmul(
                        ps[:],
                        xt[e][:, nb * 128:(nb + 1) * 128],
                        wv_t[:, e, :],
                        start=(e == 0),
                        stop=(e == 7),
                    )
                nc.vector.tensor_copy(
                    v4[:, nb, :, 0:64],
                    ps[:].rearrange("p (h d) -> p h d", h=4),
                )

            xt_cm.__exit__(None, None, None)
            pt_cm = tc.tile_pool(name="pt", bufs=14)
            pt_pool = pt_cm.__enter__()

            # ---- phase C: attention per head ----
            # Normalization is software-pipelined one (head, q-tile) unit
            # behind the chunk loop so the DVE reciprocal (the slowest DVE
            # op) never sits between the mask-adds the PE is waiting on.
            def emit_normalize(h, qt, ctx_ps):
                pair, hh = divmod(h, 2)
                hp = slice(64 * hh, 64 * hh + 64)
                craw = work_pool.tile([65, 512], F32, tag="craw", name="craw")
                nc.scalar.copy(craw[:], ctx_ps[:])
                rrec = work_pool.tile([1, 512], F32, tag="rrec", name="rrec")
                nc.vector.reciprocal(rrec[:], craw[64:65, :])
                rbr = work_pool.tile([64, 512], F32, tag="rbr", name="rbr")
                nc.gpsimd.partition_broadcast(rbr[:], rrec[:])
                nc.vector.tensor_mul(
                    ctx_pair[pair][hp, qt * 512:(qt + 1) * 512],
                    craw[0:64, :],
                    rbr[:],
                )

            # The PV matmuls for a unit are emitted while the NEXT unit's
            # S^T matmuls run, so by the time the PE (in-order) reaches a
            # PV, its exp finished long ago — the PE never drains waiting
            # on ScalarE, which would re-throttle the HAM clock gate.
            def emit_st_exp(h, qt, nchunks):
                """S^T + mask + exp for all chunks; returns PV descriptors."""
                pair, hh = divmod(h, 2)
                hp = slice(64 * hh, 64 * hh + 64)
                pv = []
                for pr in range((nchunks + 1) // 2):
                    sub = min(2, nchunks - 2 * pr)
                    st_ps = ps_st.tile([128, 2, 512], F32, tag="st")
                    offs = []
                    for s in range(sub):
                        j = 2 * pr + s
                        d = j - 4 * qt
                        # exact-causal column trim (keep matmul N >= 256)
                        off = min(128 * d, 256) if d >= 1 else 0
                        offs.append(off)
                        nc.tensor.matmul(
                            st_ps[:, s, off:],
                            kt_pair[pair][hp, j * 128:(j + 1) * 128],
                            qt_pair[pair][hp, qt * 512 + off:(qt + 1) * 512],
                            start=True,
                            stop=True,
                        )
                        if d >= 0:  # diagonal block: causal add -30000
                            u0 = 384 - 128 * d + off
                            nc.vector.tensor_tensor(
                                st_ps[:, s, off:],
                                st_ps[:, s, off:],
                                trineg_t[:, u0:u0 + 512 - off],
                                ADD,
                            )
                    pt_t = pt_pool.tile([128, 2, 512], BF16, tag="pt")
                    j0 = 2 * pr
                    if sub == 2 and offs[0] == offs[1] and j0 + 1 < jpad_min:
                        nc.scalar.activation(
                            pt_t[:, :, offs[0]:],
                            st_ps[:, :, offs[0]:],
                            EXP,
                            scale=SCALE,
                        )
                    else:
                        for s in range(sub):
                            j = j0 + s
                            kw = {}
                            if j >= jpad_min:
                                kw["bias"] = padb_t[:, j:j + 1]
                            nc.scalar.activation(
                                pt_t[:, s, offs[s]:],
                                st_ps[:, s, offs[s]:],
                                EXP,
                                scale=SCALE,
                                **kw,
                            )
                    for s in range(sub):
                        pv.append((j0 + s, pt_t, s, offs[s]))
                return pv

            def emit_pv(h, qt, nchunks, pv, ctx_ps):
                for j, pt_t, s, off in pv:
                    nc.tensor.matmul(
                        ctx_ps[:, off:],
                        v4[:, j, h, :],
                        pt_t[:, s, off:],
                        start=(j == 0),
                        stop=(j == nchunks - 1),
                        skip_group_check=True,
                    )

            units = [
                (h, qt, min(4 * qt + 4, kb_max))
                for h in range(HEADS_PER_CORE)
                for qt in range(NQT)
            ]
            prev_pv = None  # (h, qt, nchunks, pv_descs, ctx_ps)
            norm_q = []  # normalize two units behind
            for h, qt, nchunks in units:
                pv = emit_st_exp(h, qt, nchunks)
                if prev_pv is not None:
                    ph, pqt, pn, ppv, pctx = prev_pv
                    emit_pv(ph, pqt, pn, ppv, pctx)
                    norm_q.append((ph, pqt, pctx))
                if len(norm_q) > 1:
                    emit_normalize(*norm_q.pop(0))
                ctx_ps = ps_ctx.tile([65, 512], F32, tag="ctx", name="ctx")
                prev_pv = (h, qt, nchunks, pv, ctx_ps)
            ph, pqt, pn, ppv, pctx = prev_pv
            emit_pv(ph, pqt, pn, ppv, pctx)
            norm_q.append((ph, pqt, pctx))
            for u in norm_q:
                emit_normalize(*u)

            pt_cm.__exit__(None, None, None)

            # ---- phase D: output projection ----
            for nb in range(NB):
                osb = work_pool.tile([128, D], F32, tag="osb")
                for fc in range(2):
                    ps = ps_proj.tile([128, 512], F32, tag="proj")
                    for pair in range(2):
                        nc.tensor.matmul(
                            ps[:],
                            ctx_pair[pair][:, nb * 128:(nb + 1) * 128],
                            wo_t[:, pair, fc * 512:(fc + 1) * 512],
                            start=(pair == 0),
                            stop=(pair == 1),
                        )
                    nc.vector.tensor_copy(osb[:, fc * 512:(fc + 1) * 512], ps[:])
                nc.sync.dma_start(out_d[nb * 128:(nb + 1) * 128, :], osb[:])

    nc.compile()
    return nc


_PROGRAM_CACHE = {}


def kernel(x, attention_mask, W_Q, W_K, W_V, W_out, b_out):
    global LAST_RESULTS
    from concourse.bass_utils import run_bass_kernel_spmd

    x = np.ascontiguousarray(x, dtype=np.float32)
    attention_mask = np.asarray(attention_mask)
    lengths = attention_mask.astype(np.int64).sum(axis=1)
    kb_max = int(math.ceil(lengths.max() / KBLK))
    jpad_min = int(lengths.min() // KBLK)

    key = (kb_max, jpad_min)
    if key not in _PROGRAM_CACHE:
        _PROGRAM_CACHE[key] = _build_program(kb_max, jpad_min)
    nc = _PROGRAM_CACHE[key]

    # host-side input prep (matmul operands pre-cast to bf16)
    import ml_dtypes
    BF = ml_dtypes.bfloat16
    xT = [np.ascontiguousarray(x[b].T.astype(BF)) for b in range(B)]
    wqT = np.ascontiguousarray(np.asarray(W_Q, dtype=np.float32).T.astype(BF))
    wkT = np.ascontiguousarray(np.asarray(W_K, dtype=np.float32).T.astype(BF))
    wvT = np.ascontiguousarray(np.asarray(W_V, dtype=np.float32).T.astype(BF))
    woT = np.ascontiguousarray(np.asarray(W_out, dtype=np.float32).T.astype(BF))
    # padbias[p, j] = 0 if key j*128+p is real else -30000
    padb = [
        np.ascontiguousarray(
            np.where(attention_mask[b].reshape(16, 128).T != 0, 0.0, NEGB)
        ).astype(np.float32)
        for b in range(B)
    ]
    # trineg[p, u] = NEG if u < p + 384 else 0; slice [384-128d : 896-128d]
    # gives the causal additive mask for a diagonal block with offset 128d.
    pp = np.arange(128)[:, None]
    uu = np.arange(896)[None, :]
    trineg = np.where(uu < pp + 384, NEG, 0.0).astype(np.float32)
    ones65 = np.ones((128, 64), dtype=BF)

    in_maps = []
    for c in range(NCORES):
        b, g = divmod(c, 4)
        sl = slice(g * 256, (g + 1) * 256)
        in_maps.append(
            {
                "xt": xT[b],
                "wq": np.ascontiguousarray(wqT[:, sl]),
                "wk": np.ascontiguousarray(wkT[:, sl]),
                "wv": np.ascontiguousarray(wvT[:, sl]),
                "wout": np.ascontiguousarray(woT[sl, :]),
                "padbias": padb[b],
                "trineg": trineg,
                "ones65": ones65,
            }
        )

    trace = bool(int(os.environ.get("KERNEL_TRACE", "0")))
    res = run_bass_kernel_spmd(
        nc,
        in_maps,
        core_ids=list(range(NCORES)),
        trace=trace,
        trace_cores=list(range(NCORES)) if trace else None,
    )
    LAST_RESULTS = res

    out = np.zeros((B, N, D), dtype=np.float32)
    for c in range(NCORES):
        out[c // 4] += res.results[c]["out"]
    out += np.asarray(b_out, dtype=np.float32)[None, None, :]
    return out


# revision 17
# speedup vs baseline: 1.1449x; 1.0154x over previous
"""Trainium2 Bass kernel for causal+padded multi-head attention.

Problem: B=2, N=2048, D=1024, H=16 heads (DK=64), fp32 I/O.
  out = softmax(mask(x Wq^T (x Wk^T)^T) / sqrt(DK)) (x Wv^T) Wout^T + b_out

Sharding (8 cores): core c handles batch b=c//4 and heads [4*(c%4), 4*(c%4)+4).
Each core computes a partial output [N, D] (its 4 heads' contribution through
the output projection); the host sums the 4 partials per batch and adds b_out.

On-device layout (per core):
  xT   [1024, 2048]  (host-pretransposed x[b])
  QT/KT stored transposed [dk, n] as head-pair tiles [128, 2048]
  V    stored natural as [128(keys), 16 blocks, 4 heads, 65] with a ones
       column appended (col 64) so P@V' also yields the softmax denominator.
  S^T  computed per (head, q-tile 512, key-block 128) as [128, 512] in PSUM:
       matmul(lhsT=KT slice [64,128], rhs=QT slice [64,512]).
       Causal/pad masking = additive -30000 on PSUM (DVE), then one fused
       exp(0.125*s) on ScalarE straight into SBUF as float32r.
  ctx'^T [65, 512] accumulated in PSUM over key blocks:
       matmul(lhsT=V' [128,65], rhs=P^T [128,512]).
  Normalization: r = recip(rowsum) on the [1,512] denominator row,
       partition-broadcast to [64,512] (GpSimd), one DVE multiply.
  Out projection: matmul(lhsT=ctxT [128,128], rhs=WoutT [128,512]) acc over
       the two head-pair chunks.

All matmul operands are float32r (fp32 storage, ~11-bit-mantissa multiply,
full PE rate); accumulation is fp32 in PSUM.
"""

import math
import os

import numpy as np

B, N, D, H = 2, 2048, 1024, 16
DK = D // H  # 64
NCORES = 8
HEADS_PER_CORE = 4
QTILE = 512
KBLK = 128
NEG = -30000.0
NEGB = -3750.0  # pad bias applied after the 0.125 scale inside exp
SCALE = 1.0 / math.sqrt(float(DK))  # 0.125

# Set by run() when tracing is enabled (test.py reads this).
LAST_RESULTS = None


def _build_program(kb_max: int, jpad_min: int):
    import concourse.tile as tile
    from concourse import bacc, mybir

    F32 = mybir.dt.float32
    F32R = mybir.dt.float32r
    BF16 = mybir.dt.bfloat16
    EXP = mybir.ActivationFunctionType.Exp
    ADD = mybir.AluOpType.add

    nc = bacc.Bacc(None)

    xt_d = nc.dram_tensor("xt", [D, N], BF16, kind="ExternalInput")
    wq_d = nc.dram_tensor("wq", [D, 256], BF16, kind="ExternalInput")
    wk_d = nc.dram_tensor("wk", [D, 256], BF16, kind="ExternalInput")
    wv_d = nc.dram_tensor("wv", [D, 256], BF16, kind="ExternalInput")
    wout_d = nc.dram_tensor("wout", [256, D], BF16, kind="ExternalInput")
    padb_d = nc.dram_tensor("padbias", [128, 16], F32, kind="ExternalInput")
    trineg_d = nc.dram_tensor("trineg", [128, 896], F32, kind="ExternalInput")
    ones_d = nc.dram_tensor("ones65", [128, 64], BF16, kind="ExternalInput")
    out_d = nc.dram_tensor("out", [N, D], F32, kind="ExternalOutput")

    NB = N // KBLK  # 16 key/row blocks
    NQT = N // QTILE  # 4 q tiles

    with tile.TileContext(nc) as tc:
        with (
            tc.tile_pool(name="w", bufs=1) as w_pool,
            tc.tile_pool(name="big", bufs=1) as big_pool,
            tc.tile_pool(name="work", bufs=2) as work_pool,
            tc.tile_pool(name="ps_proj", bufs=2, space="PSUM") as ps_proj,
            tc.tile_pool(name="ps_st", bufs=2, space="PSUM") as ps_st,
            tc.tile_pool(name="ps_ctx", bufs=2, space="PSUM") as ps_ctx,
        ):
            # ---- load inputs ----
            xt_cm = tc.tile_pool(name="xt", bufs=8)
            xt_pool = xt_cm.__enter__()
            xt = []
            for e in range(8):
                t = xt_pool.tile([128, N], BF16, tag="xt")
                nc.sync.dma_start(t[:], xt_d[e * 128:(e + 1) * 128, :])
                xt.append(t)
            wq_t = w_pool.tile([128, 8, 256], BF16, tag="wq")
            wk_t = w_pool.tile([128, 8, 256], BF16, tag="wk")
            wv_t = w_pool.tile([128, 8, 256], BF16, tag="wv")
            wo_t = w_pool.tile([128, 2, D], BF16, tag="wo")
            nc.sync.dma_start(wq_t[:], wq_d[:].rearrange("(e p) m -> p e m", p=128))
            nc.sync.dma_start(wk_t[:], wk_d[:].rearrange("(e p) m -> p e m", p=128))
            nc.sync.dma_start(wv_t[:], wv_d[:].rearrange("(e p) m -> p e m", p=128))
            nc.sync.dma_start(wo_t[:], wout_d[:].rearrange("(c p) m -> p c m", p=128))
            padb_t = w_pool.tile([128, 16], F32, tag="padb")
            trineg_t = w_pool.tile([128, 896], F32, tag="trineg")
            nc.sync.dma_start(padb_t[:], padb_d[:])
            nc.sync.dma_start(trineg_t[:], trineg_d[:])

            # V' tile: [keys 128, key-block 16, head 4, 65]; col 64 <- ones
            v4 = big_pool.tile([128, NB, 4, 65], BF16, tag="v4")
            nc.sync.dma_start(
                v4[:, :, :, 64:65],
                ones_d[:].rearrange("p (b h o) -> p b h o", h=4, o=1),
            )

            qt_pair = [big_pool.tile([128, N], BF16, tag=f"qt{p}", name=f"qt{p}") for p in range(2)]
            kt_pair = [big_pool.tile([128, N], BF16, tag=f"kt{p}", name=f"kt{p}") for p in range(2)]
            ctx_pair = [big_pool.tile([128, N], BF16, tag=f"ctx{p}", name=f"ctx{p}") for p in range(2)]

            # ---- phase B: projections ----
            # QT/KT: [dk(128 = 2 heads), n] = (W.T chunk)^T @ xT
            for name, w_t, dst in (("q", wq_t, qt_pair), ("k", wk_t, kt_pair)):
                for pair in range(2):
                    for nq in range(NQT):
                        ps = ps_proj.tile([128, 512], F32, tag="proj")
                        for e in range(8):
                            nc.tensor.matmul(
                                ps[:],
                                wq_t[:, e, pair * 128:(pair + 1) * 128]
                                if name == "q"
                                else wk_t[:, e, pair * 128:(pair + 1) * 128],
                                xt[e][:, nq * 512:(nq + 1) * 512],
                                start=(e == 0),
                                stop=(e == 7),
                            )
                        nc.vector.tensor_copy(
                            dst[pair][:, nq * 512:(nq + 1) * 512], ps[:]
                        )
            # V natural: [n-block, 4*64] = xT-chunk^T @ WvT-chunk
            for nb in range(NB):
                ps = ps_proj.tile([128, 256], F32, tag="proj")
                for e in range(8):
                    nc.tensor.matmul(
                        ps[:],
                        xt[e][:, nb * 128:(nb + 1) * 128],
                        wv_t[:, e, :],
                        start=(e == 0),
                        stop=(e == 7),
                    )
                nc.vector.tensor_copy(
                    v4[:, nb, :, 0:64],
                    ps[:].rearrange("p (h d) -> p h d", h=4),
                )

            xt_cm.__exit__(None, None, None)
            pt_cm = tc.tile_pool(name="pt", bufs=14)
            pt_pool = pt_cm.__enter__()

            # ---- phase C: attention per head ----
            # Normalization is software-pipelined one (head, q-tile) unit
            # behind the chunk loop so the DVE reciprocal (the slowest DVE
            # op) never sits between the mask-adds the PE is waiting on.
            def emit_normalize(h, qt, ctx_ps):
                pair, hh = divmod(h, 2)
                hp = slice(64 * hh, 64 * hh + 64)
                craw = work_pool.tile([65, 512], F32, tag="craw", name="craw")
                nc.scalar.copy(craw[:], ctx_ps[:])
                rrec = work_pool.tile([1, 512], F32, tag="rrec", name="rrec")
                nc.vector.reciprocal(rrec[:], craw[64:65, :])
                rbr = work_pool.tile([64, 512], F32, tag="rbr", name="rbr")
                nc.gpsimd.partition_broadcast(rbr[:], rrec[:])
                nc.vector.tensor_mul(
                    ctx_pair[pair][hp, qt * 512:(qt + 1) * 512],
                    craw[0:64, :],
                    rbr[:],
                )

            # The PV matmuls for a unit are emitted while the NEXT unit's
            # S^T matmuls run, so by the time the PE (in-order) reaches a
            # PV, its exp finished long ago — the PE never drains waiting
            # on ScalarE, which would re-throttle the HAM clock gate.
            def emit_st_exp(h, qt, nchunks):
                """S^T + mask + exp for all chunks; returns PV descriptors."""
                pair, hh = divmod(h, 2)
                hp = slice(64 * hh, 64 * hh + 64)
                pv = []
                for pr in range((nchunks + 1) // 2):
                    sub = min(2, nchunks - 2 * pr)
                    st_ps = ps_st.tile([128, 2, 512], F32, tag="st")
                    offs = []
                    for s in range(sub):
                        j = 2 * pr + s
                        d = j - 4 * qt
                        # exact-causal column trim (keep matmul N >= 256)
                        off = min(128 * d, 256) if d >= 1 else 0
                        offs.append(off)
                        nc.tensor.matmul(
                            st_ps[:, s, off:],
                            kt_pair[pair][hp, j * 128:(j + 1) * 128],
                            qt_pair[pair][hp, qt * 512 + off:(qt + 1) * 512],
                            start=True,
                            stop=True,
                        )
                        if d >= 0:  # diagonal block: causal add -30000
                            u0 = 384 - 128 * d + off
                            nc.vector.tensor_tensor(
                                st_ps[:, s, off:],
                                st_ps[:, s, off:],
                                trineg_t[:, u0:u0 + 512 - off],
                                ADD,
                            )
                    pt_t = pt_pool.tile([128, 2, 512], BF16, tag="pt")
                    j0 = 2 * pr
                    if sub == 2 and offs[0] == offs[1] and j0 + 1 < jpad_min:
                        nc.scalar.activation(
                            pt_t[:, :, offs[0]:],
                            st_ps[:, :, offs[0]:],
                            EXP,
                            scale=SCALE,
                        )
                    else:
                        for s in range(sub):
                            j = j0 + s
                            kw = {}
                            if j >= jpad_min:
                                kw["bias"] = padb_t[:, j:j + 1]
                            nc.scalar.activation(
                                pt_t[:, s, offs[s]:],
                                st_ps[:, s, offs[s]:],
                                EXP,
                                scale=SCALE,
                                **kw,
                            )
                    for s in range(sub):
                        pv.append((j0 + s, pt_t, s, offs[s]))
                return pv

            def emit_pv(h, qt, nchunks, pv, ctx_ps):
                for j, pt_t, s, off in pv:
                    nc.tensor.matmul(
                        ctx_ps[:, off:],
                        v4[:, j, h, :],
                        pt_t[:, s, off:],
                        start=(j == 0),
                        stop=(j == nchunks - 1),
                        skip_group_check=True,
                    )

            units = [
                (h, qt, min(4 * qt + 4, kb_max))
                for h in range(HEADS_PER_CORE)
                for qt in range(NQT)
            ]
            prev_pv = None  # (h, qt, nchunks, pv_descs, ctx_ps)
            norm_q = []  # normalize two units behind
            for h, qt, nchunks in units:
                pv = emit_st_exp(h, qt, nchunks)
                if prev_pv is not None:
                    ph, pqt, pn, ppv, pctx = prev_pv
                    emit_pv(ph, pqt, pn, ppv, pctx)
                    norm_q.append((ph, pqt, pctx))
                if len(norm_q) > 1:
                    emit_normalize(*norm_q.pop(0))
                ctx_ps = ps_ctx.tile([65, 512], F32, tag="ctx", name="ctx")
                prev_pv = (h, qt, nchunks, pv, ctx_ps)
            ph, pqt, pn, ppv, pctx = prev_pv
            emit_pv(ph, pqt, pn, ppv, pctx)
            norm_q.append((ph, pqt, pctx))
            for u in norm_q:
                emit_normalize(*u)

            pt_cm.__exit__(None, None, None)

            # ---- phase D: output projection ----
            for nb in range(NB):
                osb = work_pool.tile([128, D], F32, tag="osb")
                for fc in range(2):
                    ps = ps_proj.tile([128, 512], F32, tag="proj")
                    for pair in range(2):
                        nc.tensor.matmul(
                            ps[:],
                            ctx_pair[pair][:, nb * 128:(nb + 1) * 128],
                            wo_t[:, pair, fc * 512:(fc + 1) * 512],
                            start=(pair == 0),
                            stop=(pair == 1),
                        )
                    nc.vector.tensor_copy(osb[:, fc * 512:(fc + 1) * 512], ps[:])
                nc.sync.dma_start(out_d[nb * 128:(nb + 1) * 128, :], osb[:])

    nc.compile()
    return nc


_PROGRAM_CACHE = {}


def kernel(x, attention_mask, W_Q, W_K, W_V, W_out, b_out):
    global LAST_RESULTS
    from concourse.bass_utils import run_bass_kernel_spmd

    x = np.ascontiguousarray(x, dtype=np.float32)
    attention_mask = np.asarray(attention_mask)
    lengths = attention_mask.astype(np.int64).sum(axis=1)
    kb_max = int(math.ceil(lengths.max() / KBLK))
    jpad_min = int(lengths.min() // KBLK)

    key = (kb_max, jpad_min)
    if key not in _PROGRAM_CACHE:
        _PROGRAM_CACHE[key] = _build_program(kb_max, jpad_min)
    nc = _PROGRAM_CACHE[key]

    # host-side input prep (matmul operands pre-cast to bf16)
    import ml_dtypes
    BF = ml_dtypes.bfloat16
    xT = [np.ascontiguousarray(x[b].T.astype(BF)) for b in range(B)]
    wqT = np.ascontiguousarray(np.asarray(W_Q, dtype=np.float32).T.astype(BF))
    wkT = np.ascontiguousarray(np.asarray(W_K, dtype=np.float32).T.astype(BF))
    wvT = np.ascontiguousarray(np.asarray(W_V, dtype=np.float32).T.astype(BF))
    woT = np.ascontiguousarray(np.asarray(W_out, dtype=np.float32).T.astype(BF))
    # padbias[p, j] = 0 if key j*128+p is real else -30000
    padb = [
        np.ascontiguousarray(
            np.where(attention_mask[b].reshape(16, 128).T != 0, 0.0, NEGB)
        ).astype(np.float32)
        for b in range(B)
    ]
    # trineg[p, u] = NEG if u < p + 384 else 0; slice [384-128d : 896-128d]
    # gives the causal additive mask for a diagonal block with offset 128d.
    pp = np.arange(128)[:, None]
    uu = np.arange(896)[None, :]
    trineg = np.where(uu < pp + 384, NEG, 0.0).astype(np.float32)
    ones65 = np.ones((128, 64), dtype=BF)

    in_maps = []
    for c in range(NCORES):
        b, g = divmod(c, 4)
        sl = slice(g * 256, (g + 1) * 256)
        in_maps.append(
            {
                "xt": xT[b],
                "wq": np.ascontiguousarray(wqT[:, sl]),
                "wk": np.ascontiguousarray(wkT[:, sl]),
                "wv": np.ascontiguousarray(wvT[:, sl]),
                "wout": np.ascontiguousarray(woT[sl, :]),
                "padbias": padb[b],
                "trineg": trineg,
                "ones65": ones65,
            }
        )

    trace = bool(int(os.environ.get("KERNEL_TRACE", "0")))
    ncores_run = int(os.environ.get("KERNEL_NCORES", str(NCORES)))
    res = run_bass_kernel_spmd(
        nc,
        in_maps[:ncores_run],
        core_ids=list(range(ncores_run)),
        trace=trace,
        trace_cores=list(range(ncores_run)) if trace else None,
    )
    LAST_RESULTS = res

    out = np.zeros((B, N, D), dtype=np.float32)
    for c in range(len(res.results)):
        out[c // 4] += res.results[c]["out"]
    out += np.asarray(b_out, dtype=np.float32)[None, None, :]
    return out


# revision 19
# speedup vs baseline: 1.2509x; 1.0925x over previous
"""Trainium2 Bass kernel for causal+padded multi-head attention.

Problem: B=2, N=2048, D=1024, H=16 heads (DK=64), fp32 I/O.
  out = softmax(mask(x Wq^T (x Wk^T)^T) / sqrt(DK)) (x Wv^T) Wout^T + b_out

Sharding (8 cores): core c handles batch b=c//4 and heads [4*(c%4), 4*(c%4)+4).
Each core computes a partial output [N, D] (its 4 heads' contribution through
the output projection); the host sums the 4 partials per batch and adds b_out.

On-device layout (per core):
  xT   [1024, 2048]  (host-pretransposed x[b])
  QT/KT stored transposed [dk, n] as head-pair tiles [128, 2048]
  V    stored natural as [128(keys), 16 blocks, 4 heads, 65] with a ones
       column appended (col 64) so P@V' also yields the softmax denominator.
  S^T  computed per (head, q-tile 512, key-block 128) as [128, 512] in PSUM:
       matmul(lhsT=KT slice [64,128], rhs=QT slice [64,512]).
       Causal/pad masking = additive -30000 on PSUM (DVE), then one fused
       exp(0.125*s) on ScalarE straight into SBUF as float32r.
  ctx'^T [65, 512] accumulated in PSUM over key blocks:
       matmul(lhsT=V' [128,65], rhs=P^T [128,512]).
  Normalization: r = recip(rowsum) on the [1,512] denominator row,
       partition-broadcast to [64,512] (GpSimd), one DVE multiply.
  Out projection: matmul(lhsT=ctxT [128,128], rhs=WoutT [128,512]) acc over
       the two head-pair chunks.

All matmul operands are float32r (fp32 storage, ~11-bit-mantissa multiply,
full PE rate); accumulation is fp32 in PSUM.
"""

import math
import os

import numpy as np

B, N, D, H = 2, 2048, 1024, 16
DK = D // H  # 64
NCORES = 8
HEADS_PER_CORE = 4
QTILE = 512
KBLK = 128
NEG = -30000.0
NEGB = -3750.0  # pad bias applied after the 0.125 scale inside exp
SCALE = 1.0 / math.sqrt(float(DK))  # 0.125

# Set by run() when tracing is enabled (test.py reads this).
LAST_RESULTS = None


def _build_program(kb_max: int, jpad_min: int):
    import concourse.tile as tile
    from concourse import bacc, mybir

    F32 = mybir.dt.float32
    F32R = mybir.dt.float32r
    BF16 = mybir.dt.bfloat16
    EXP = mybir.ActivationFunctionType.Exp
    ADD = mybir.AluOpType.add

    nc = bacc.Bacc(None)

    xt_d = nc.dram_tensor("xt", [D, N], BF16, kind="ExternalInput")
    wq_d = nc.dram_tensor("wq", [D, 256], BF16, kind="ExternalInput")
    wk_d = nc.dram_tensor("wk", [D, 256], BF16, kind="ExternalInput")
    wv_d = nc.dram_tensor("wv", [D, 256], BF16, kind="ExternalInput")
    wout_d = nc.dram_tensor("wout", [256, D], BF16, kind="ExternalInput")
    padb_d = nc.dram_tensor("padbias", [128, 16], F32, kind="ExternalInput")
    trineg_d = nc.dram_tensor("trineg", [128, 896], F32, kind="ExternalInput")
    ones_d = nc.dram_tensor("ones65", [128, 64], BF16, kind="ExternalInput")
    out_d = nc.dram_tensor("out", [N, D], F32, kind="ExternalOutput")

    NB = N // KBLK  # 16 key/row blocks
    NQT = N // QTILE  # 4 q tiles

    with tile.TileContext(nc) as tc:
        with (
            tc.tile_pool(name="w", bufs=1) as w_pool,
            tc.tile_pool(name="big", bufs=1) as big_pool,
            tc.tile_pool(name="work", bufs=2) as work_pool,
            tc.tile_pool(name="ps_proj", bufs=2, space="PSUM") as ps_proj,
            tc.tile_pool(name="ps_st", bufs=2, space="PSUM") as ps_st,
            tc.tile_pool(name="ps_ctx", bufs=1, space="PSUM") as ps_ctx,
        ):
            # ---- load inputs ----
            xt_cm = tc.tile_pool(name="xt", bufs=8)
            xt_pool = xt_cm.__enter__()
            xt = []
            for e in range(8):
                t = xt_pool.tile([128, N], BF16, tag="xt")
                nc.sync.dma_start(t[:], xt_d[e * 128:(e + 1) * 128, :])
                xt.append(t)
            wq_t = w_pool.tile([128, 8, 256], BF16, tag="wq")
            wk_t = w_pool.tile([128, 8, 256], BF16, tag="wk")
            wv_t = w_pool.tile([128, 8, 256], BF16, tag="wv")
            wo_t = w_pool.tile([128, 2, D], BF16, tag="wo")
            nc.sync.dma_start(wq_t[:], wq_d[:].rearrange("(e p) m -> p e m", p=128))
            nc.sync.dma_start(wk_t[:], wk_d[:].rearrange("(e p) m -> p e m", p=128))
            nc.sync.dma_start(wv_t[:], wv_d[:].rearrange("(e p) m -> p e m", p=128))
            nc.sync.dma_start(wo_t[:], wout_d[:].rearrange("(c p) m -> p c m", p=128))
            padb_t = w_pool.tile([128, 16], F32, tag="padb")
            trineg_t = w_pool.tile([128, 896], F32, tag="trineg")
            nc.sync.dma_start(padb_t[:], padb_d[:])
            nc.sync.dma_start(trineg_t[:], trineg_d[:])

            # V' tile: [keys 128, key-block 16, head 4, 65]; col 64 <- ones
            v4 = big_pool.tile([128, NB, 4, 65], BF16, tag="v4")
            nc.sync.dma_start(
                v4[:, :, :, 64:65],
                ones_d[:].rearrange("p (b h o) -> p b h o", h=4, o=1),
            )

            qt_pair = [big_pool.tile([128, N], BF16, tag=f"qt{p}", name=f"qt{p}") for p in range(2)]
            kt_pair = [big_pool.tile([128, N], BF16, tag=f"kt{p}", name=f"kt{p}") for p in range(2)]
            ctx_pair = [big_pool.tile([128, N], BF16, tag=f"ctx{p}", name=f"ctx{p}") for p in range(2)]

            # ---- phase B: projections ----
            # QT/KT: [dk(128 = 2 heads), n] = (W.T chunk)^T @ xT
            for name, w_t, dst in (("q", wq_t, qt_pair), ("k", wk_t, kt_pair)):
                for pair in range(2):
                    for nq in range(NQT):
                        ps = ps_proj.tile([128, 512], F32, tag="proj")
                        for e in range(8):
                            nc.tensor.matmul(
                                ps[:],
                                wq_t[:, e, pair * 128:(pair + 1) * 128]
                                if name == "q"
                                else wk_t[:, e, pair * 128:(pair + 1) * 128],
                                xt[e][:, nq * 512:(nq + 1) * 512],
                                start=(e == 0),
                                stop=(e == 7),
                            )
                        nc.vector.tensor_copy(
                            dst[pair][:, nq * 512:(nq + 1) * 512], ps[:]
                        )
            # V natural: [n-block, 4*64] = xT-chunk^T @ WvT-chunk
            for nb in range(NB):
                ps = ps_proj.tile([128, 256], F32, tag="proj")
                for e in range(8):
                    nc.tensor.matmul(
                        ps[:],
                        xt[e][:, nb * 128:(nb + 1) * 128],
                        wv_t[:, e, :],
                        start=(e == 0),
                        stop=(e == 7),
                    )
                nc.vector.tensor_copy(
                    v4[:, nb, :, 0:64],
                    ps[:].rearrange("p (h d) -> p h d", h=4),
                )

            xt_cm.__exit__(None, None, None)
            pt_cm = tc.tile_pool(name="pt", bufs=26)
            pt_pool = pt_cm.__enter__()

            # ---- phase C: attention, head pairs interleaved ----
            # A unit is (head-pair, q-tile). The two heads' S^T matmuls sit
            # at base partitions 0 / 64 (row groups 0-63 / 64-127), so they
            # execute concurrently on the PE and their weight loads overlap
            # the other head's matmul — no LDW bubble, HAM stays warm.
            # PV matmuls run one unit behind their exps so the in-order PE
            # never drains waiting on ScalarE.
            def emit_normalize(pair, hh, qt, ctx_ps):
                hp = slice(64 * hh, 64 * hh + 64)
                craw = work_pool.tile([65, 512], F32, tag="craw", name="craw")
                nc.scalar.copy(craw[:], ctx_ps[:])
                rrec = work_pool.tile([1, 512], F32, tag="rrec", name="rrec")
                nc.vector.reciprocal(rrec[:], craw[64:65, :])
                rbr = work_pool.tile([64, 512], F32, tag="rbr", name="rbr")
                nc.gpsimd.partition_broadcast(rbr[:], rrec[:])
                nc.vector.tensor_mul(
                    ctx_pair[pair][hp, qt * 512:(qt + 1) * 512],
                    craw[0:64, :],
                    rbr[:],
                )

            def emit_st_exp(pair, qt, nchunks):
                """S^T + mask + exp for both heads; returns PV descriptors."""
                pv = []
                for j in range(nchunks):
                    d = j - 4 * qt
                    # exact-causal column trim (keep matmul N >= 256)
                    off = min(128 * d, 256) if d >= 1 else 0
                    st_ps = ps_st.tile([128, 2, 512], F32, tag="st")
                    for hh in range(2):
                        hp = slice(64 * hh, 64 * hh + 64)
                        nc.tensor.matmul(
                            st_ps[:, hh, off:],
                            kt_pair[pair][hp, j * 128:(j + 1) * 128],
                            qt_pair[pair][hp, qt * 512 + off:(qt + 1) * 512],
                            start=True,
                            stop=True,
                        )
                    if d >= 0:  # diagonal block: causal add -30000
                        u0 = 384 - 128 * d + off
                        for hh in range(2):
                            nc.vector.tensor_tensor(
                                st_ps[:, hh, off:],
                                st_ps[:, hh, off:],
                                trineg_t[:, u0:u0 + 512 - off],
                                ADD,
                            )
                    pt_t = pt_pool.tile([128, 2, 512], BF16, tag="pt")
                    kw = {}
                    if j >= jpad_min:  # per-key pad bias (same for both heads)
                        kw["bias"] = padb_t[:, j:j + 1]
                    nc.scalar.activation(
                        pt_t[:, :, off:], st_ps[:, :, off:], EXP, scale=SCALE, **kw
                    )
                    pv.append((j, pt_t, off))
                return pv

            def emit_pv(pair, qt, nchunks, pv, ctx2):
                for j, pt_t, off in pv:
                    for hh in range(2):
                        nc.tensor.matmul(
                            ctx2[hh][:, off:],
                            v4[:, j, 2 * pair + hh, :],
                            pt_t[:, hh, off:],
                            start=(j == 0),
                            stop=(j == nchunks - 1),
                            skip_group_check=True,
                        )

            units = [
                (pair, qt, min(4 * qt + 4, kb_max))
                for pair in range(2)
                for qt in range(NQT)
            ]
            prev_pv = None  # (pair, qt, nchunks, pv_descs, ctx2)
            norm_q = []  # normalize one unit behind the PV
            for pair, qt, nchunks in units:
                pv = emit_st_exp(pair, qt, nchunks)
                if prev_pv is not None:
                    ppair, pqt, pn, ppv, pctx2 = prev_pv
                    emit_pv(ppair, pqt, pn, ppv, pctx2)
                    norm_q.append((ppair, pqt, pctx2))
                if len(norm_q) > 1:
                    npair, nqt, nctx2 = norm_q.pop(0)
                    for hh in range(2):
                        emit_normalize(npair, hh, nqt, nctx2[hh])
                ctx2 = [
                    ps_ctx.tile([65, 512], F32, tag=f"ctx{hh}", name=f"ctx{hh}")
                    for hh in range(2)
                ]
                prev_pv = (pair, qt, nchunks, pv, ctx2)
            ppair, pqt, pn, ppv, pctx2 = prev_pv
            emit_pv(ppair, pqt, pn, ppv, pctx2)
            norm_q.append((ppair, pqt, pctx2))
            for npair, nqt, nctx2 in norm_q:
                for hh in range(2):
                    emit_normalize(npair, hh, nqt, nctx2[hh])

            pt_cm.__exit__(None, None, None)

            # ---- phase D: output projection ----
            for nb in range(NB):
                osb = work_pool.tile([128, D], F32, tag="osb")
                for fc in range(2):
                    ps = ps_proj.tile([128, 512], F32, tag="proj")
                    for pair in range(2):
                        nc.tensor.matmul(
                            ps[:],
                            ctx_pair[pair][:, nb * 128:(nb + 1) * 128],
                            wo_t[:, pair, fc * 512:(fc + 1) * 512],
                            start=(pair == 0),
                            stop=(pair == 1),
                        )
                    nc.vector.tensor_copy(osb[:, fc * 512:(fc + 1) * 512], ps[:])
                nc.sync.dma_start(out_d[nb * 128:(nb + 1) * 128, :], osb[:])

    nc.compile()
    return nc


_PROGRAM_CACHE = {}


def kernel(x, attention_mask, W_Q, W_K, W_V, W_out, b_out):
    global LAST_RESULTS
    from concourse.bass_utils import run_bass_kernel_spmd

    x = np.ascontiguousarray(x, dtype=np.float32)
    attention_mask = np.asarray(attention_mask)
    lengths = attention_mask.astype(np.int64).sum(axis=1)
    kb_max = int(math.ceil(lengths.max() / KBLK))
    jpad_min = int(lengths.min() // KBLK)

    key = (kb_max, jpad_min)
    if key not in _PROGRAM_CACHE:
        _PROGRAM_CACHE[key] = _build_program(kb_max, jpad_min)
    nc = _PROGRAM_CACHE[key]

    # host-side input prep (matmul operands pre-cast to bf16)
    import ml_dtypes
    BF = ml_dtypes.bfloat16
    xT = [np.ascontiguousarray(x[b].T.astype(BF)) for b in range(B)]
    wqT = np.ascontiguousarray(np.asarray(W_Q, dtype=np.float32).T.astype(BF))
    wkT = np.ascontiguousarray(np.asarray(W_K, dtype=np.float32).T.astype(BF))
    wvT = np.ascontiguousarray(np.asarray(W_V, dtype=np.float32).T.astype(BF))
    woT = np.ascontiguousarray(np.asarray(W_out, dtype=np.float32).T.astype(BF))
    # padbias[p, j] = 0 if key j*128+p is real else -30000
    padb = [
        np.ascontiguousarray(
            np.where(attention_mask[b].reshape(16, 128).T != 0, 0.0, NEGB)
        ).astype(np.float32)
        for b in range(B)
    ]
    # trineg[p, u] = NEG if u < p + 384 else 0; slice [384-128d : 896-128d]
    # gives the causal additive mask for a diagonal block with offset 128d.
    pp = np.arange(128)[:, None]
    uu = np.arange(896)[None, :]
    trineg = np.where(uu < pp + 384, NEG, 0.0).astype(np.float32)
    ones65 = np.ones((128, 64), dtype=BF)

    in_maps = []
    for c in range(NCORES):
        b, g = divmod(c, 4)
        sl = slice(g * 256, (g + 1) * 256)
        in_maps.append(
            {
                "xt": xT[b],
                "wq": np.ascontiguousarray(wqT[:, sl]),
                "wk": np.ascontiguousarray(wkT[:, sl]),
                "wv": np.ascontiguousarray(wvT[:, sl]),
                "wout": np.ascontiguousarray(woT[sl, :]),
                "padbias": padb[b],
                "trineg": trineg,
                "ones65": ones65,
            }
        )

    trace = bool(int(os.environ.get("KERNEL_TRACE", "0")))
    ncores_run = int(os.environ.get("KERNEL_NCORES", str(NCORES)))
    res = run_bass_kernel_spmd(
        nc,
        in_maps[:ncores_run],
        core_ids=list(range(ncores_run)),
        trace=trace,
        trace_cores=list(range(ncores_run)) if trace else None,
    )
    LAST_RESULTS = res

    out = np.zeros((B, N, D), dtype=np.float32)
    for c in range(len(res.results)):
        out[c // 4] += res.results[c]["out"]
    out += np.asarray(b_out, dtype=np.float32)[None, None, :]
    return out
